# revision 11
# baseline (speedup 1.0000x reference)
"""Trainium2 Bass kernel for nn_EquiCtsConvBase (equivariant continuous conv).

Math (per batch b, center m, field point n):
  rel = (field[n] - center[m]) / RADIUS
  r, theta = polar(rel)
  Bilinear grid-sample of kernel[(co,ci,y,x), theta_pad, r] decomposes into
  separable hats:
    Wx[j]  = relu(1 - |4r - 0.5 - j|)                 j = 0..3
    Wy8[b] = relu(1 - dwrap(iy - (b+1)))              b = 0..7
             iy = 4*theta/pi + 4.5, dwrap(u) = min(|u|, 8-|u|)  (period-8)
  (dwrap reproduces the reference's circular theta padding fold exactly.)
  att = relu(1 - |rel|^2)^3 * mask[n]
  A[(b,j), n, m] = Wx[j] * (Wy8[b] * att)
  G[f, (cell, m)] = sum_n feat[n, f] * A[cell, n, m]      (PE matmul 1, fp16)
  out[oy, m]      = sum_{cell,f} K2[cell, f, oy] * G[f, (cell,m)]  (PE mm 2)
  out /= psi;  psi[m] = sum_n att  (feat pad columns are mask => psi lands on
  PSUM partitions 32..47 of the att column, one per oy partition)

theta without Sqrt (single ACT table, trig_and_small):
  t = y/x (via fast reciprocal); swap to x/y when |t| > 1
  phi = arctan(t_sel); fixed to sign(t)*pi/2 - phi in the swap branch
  r = |x*sin(phi+pi/2) + y*sin(phi)|

Heavy per-(m,n) chains run as single custom DVE ops (registered at runtime
via the dve_ops extension API); hat/att outputs are fp16 so the 32 A-cell
products run in DVE 2x 16-bit mode and both matmuls run at full fp16 rate.

Sharding: 8 cores; core c handles batch b = c//4, centers m0 = (c%4)*96..+96.
"""

import math
import numpy as np

RADIUS = 1.5
B, M, N = 2, 384, 384
CI = CO = 8
M_LOC = 96          # centers per core
NCH = 3             # n-chunks of 128 (N = 384)
NCELL = 32          # 8 theta bins x 4 radius cells
FREE = NCH * M_LOC  # 288 fused (chunk, m) free dim
N_CORES = 8

CFG = dict(
    pool_j=(3,),         # which j-columns of A-cell products go to Pool
    wy_custom=(0, 7),    # bins via custom wrap-hat DVE op (rest: 2 ACT ops)
    pe_warm=6,           # tensor-engine warm-up matmuls during phase 1
    gs_copy_engs="SVSVSVS",
)

_module_cache = {}
_custom_ops = {}


def _register_custom_ops():
    """Register fused DVE ops via the dve_ops extension API (runtime append,
    same contract as editing dve_ops.py: sha pinned from lower() output)."""
    if _custom_ops:
        return _custom_ops
    import concourse.dve_ops as dvo
    from concourse.dve_ops import DveOp
    from concourse.dve_spec import (
        Spec, Src0, Src1, C0, C1, C2, Zero, One, relu, maxx, minn, select,
        lower, AluOp, Bin, _has_src1,
    )
    from concourse.dve_uop import DveOpSpec

    def _np32(x):
        return np.asarray(x, np.float32)

    def mk(name, body, ref):
        existing = {op.name: op for op in dvo.OPS}
        if name in existing:
            _custom_ops[name] = existing[name]
            return
        spec = Spec(body=body, reference=ref)
        row = dvo._CUSTOM_DVE_ROW_BASE + len(dvo.OPS)
        assert row < 0x20, "custom-DVE row space exhausted"
        shas = {}
        for ver in ("v3", "v4"):
            uops = lower(spec, ver=ver)
            shas[ver] = DveOpSpec(
                name=name, opcode=row, uops=uops, rd1_en=_has_src1(spec)
            ).sha(ver)
        op = DveOp(name, spec, subdim=False, uops_sha=shas)
        dvo.OPS.append(op)
        dvo.CUSTOM_DVE_SPECS[name] = spec
        dvo._SUB_OPCODE_FOR_NAME[name] = row
        _custom_ops[name] = op

    # phi = (phi1^2 > (pi/4)^2) ? copysign(pi/2, phi1) - phi2 : phi1
    # (Src0 = arctan(t1), Src1 = arctan(1/t1); C0 = pi/2, C1 = (pi/4)^2)
    c = C1 < Src0 * Src0
    s = select(Src0 < Zero, Zero - C0, C0)
    mk("EQ_PHI_MERGE", select(c, s - Src1, Src0),
       lambda in0, in1, s0, s1, imm2:
           np.where(_np32(in0) * in0 > np.float32(s1),
                    np.copysign(np.float32(s0), _np32(in0)) - _np32(in1),
                    _np32(in0)))

    # corr = copysign(1, rely) * (relx < 0)   (Src0=rely, Src1=relx)
    ones = select(Src0 < Zero, Zero - One, One)
    mk("EQ_CORR", ones * (Src1 < Zero),
       lambda in0, in1, s0, s1, imm2:
           np.copysign(np.float32(1), _np32(in0)) * (_np32(in1) < 0))

    # iy = phi*C0 + C1 + corr*C2
    mk("EQ_IY_AFF", Src0 * C0 + C1 + Src1 * C2,
       lambda in0, in1, s0, s1, imm2:
           _np32(in0) * np.float32(s0) + np.float32(s1)
           + _np32(in1) * np.float32(imm2))

    # att = relu(1 - (sqx + sqy))^3   (Src0=sqx, Src1=sqy)
    r2 = relu(One - (Src0 + Src1))
    mk("EQ_ATT_CUBE2", (r2 * r2) * r2,
       lambda in0, in1, s0, s1, imm2:
           np.maximum(1.0 - (_np32(in0) + _np32(in1)), 0.0) ** 3)

    # pure wrap hat: relu(1 - min(|iy + C0|, C1 - |iy + C0|))  (C1 = 8)
    y = Src0 + C0
    ay = maxx(y, Zero - y)
    d = minn(ay, C1 - ay)
    mk("EQ_WRAP_HAT_P", relu(One - d),
       lambda in0, in1, s0, s1, imm2:
           np.maximum(1.0 - np.minimum(np.abs(_np32(in0) + s0),
                                       s1 - np.abs(_np32(in0) + s0)), 0.0))

    # wxa_j = relu(1 - |r*C0 + C1|) * att   (att folded into the x-hats)
    u = Src0 * C0 + C1
    mk("EQ_WX_HAT_A", maxx(minn(One - u, One + u), Zero) * Src1,
       lambda in0, in1, s0, s1, imm2:
           np.maximum(1.0 - np.abs(_np32(in0) * s0 + s1), 0.0) * _np32(in1))

    # rr = |xc + ys|
    s2 = Src0 + Src1
    mk("EQ_RR_ABS", maxx(s2, Zero - s2),
       lambda in0, in1, s0, s1, imm2: np.abs(_np32(in0) + _np32(in1)))

    return _custom_ops


def _build_module(cfg):
    import concourse.bass as bass
    import concourse.bacc as bacc
    import concourse.mybir as mybir
    from concourse import tile

    ops = _register_custom_ops()

    dt = mybir.dt
    Alu = mybir.AluOpType
    Act = mybir.ActivationFunctionType

    nc = bacc.Bacc("TRN2", target_bir_lowering=False, debug=False,
                   num_devices=N_CORES)

    # Pre-register ACT bias constants (memset + barrier) so ACT ops don't
    # need a DMA sync wait.
    _eng_rr = [nc.gpsimd, nc.vector]

    def _register_const(value):
        key = (dt.float32, float(value))
        if key in nc.const_aps.aps:
            return
        t = nc.alloc_sbuf_tensor(
            f"kcst-{len(nc.const_aps.aps)}", [128, 1], dt.float32)
        _eng_rr[len(nc.const_aps.aps) % 2].memset(t.ap(), float(value))
        nc.const_aps.aps[key] = t.ap()

    for _v in [-2.0, -3.0, -4.0, -5.0, -6.0, -7.0, math.pi / 2, 1.0]:
        _register_const(_v)
    nc.all_engine_barrier()

    # ------------- DRAM I/O -------------
    # inp cols: 0..95 cx, 96..191 cy (/RADIUS, per m), 192..197 fx|fy per
    # chunk, 198..213 bias constants
    inpd = nc.dram_tensor("inp", [128, 214], dt.float32,
                          kind="ExternalInput").ap()
    featd = nc.dram_tensor("featx", [128, NCH * 48], dt.float16,
                           kind="ExternalInput").ap()
    k2d = nc.dram_tensor("k2b", [16, NCELL * 16], dt.float16,
                         kind="ExternalInput").ap()
    outd = nc.dram_tensor("out", [16, M_LOC], dt.float32,
                          kind="ExternalOutput").ap()

    f32 = dt.float32
    f16 = dt.float16
    V, S, G = nc.vector, nc.scalar, nc.gpsimd
    CD = nc.vector._custom_dve

    with tile.TileContext(nc) as tc:
        with tc.tile_pool(name="p", bufs=1) as pool, \
             tc.tile_pool(name="ps", bufs=1, space="PSUM") as psum:

            # ---------- loads ----------
            inp_s = pool.tile([128, 214], f32, tag="inp", name="inp_s")
            feat_s = pool.tile([128, NCH * 48], f16, tag="feat", name="feat_s")
            k2_s = pool.tile([16, NCELL * 16], f16, tag="k2", name="k2_s")
            nc.sync.dma_start(inp_s[:], inpd[:])
            nc.sync.dma_start(feat_s[:], featd[:])
            nc.sync.dma_start(k2_s[:], k2d[:])

            def wt(tag, shape=None, dtp=f32):
                return pool.tile(shape or [128, FREE], dtp, tag=tag, name=tag)

            # broadcast views [128, NCH, M_LOC]
            cb_x = inp_s[:, None, 0:M_LOC].to_broadcast((128, NCH, M_LOC))
            cb_y = inp_s[:, None, M_LOC:2 * M_LOC].to_broadcast(
                (128, NCH, M_LOC))
            fx_b = inp_s[:, 192:195, None].to_broadcast((128, NCH, M_LOC))
            fy_b = inp_s[:, 195:198, None].to_broadcast((128, NCH, M_LOC))

            # warm-up: pin the trig_and_small ACT table
            warm = pool.tile([1, 1], f32, tag="warm", name="warm")
            zc = nc.const_aps.aps[(dt.float32, 0.0)][0:1]
            S.activation(warm[:], zc, Act.Sin)
            S.activation(warm[:], zc, Act.Arctan)

            # ---------- elementwise ----------
            rel = pool.tile([128, 2, NCH, M_LOC], f32, tag="rel", name="rel")
            sq2 = pool.tile([128, 2, FREE], f32, tag="sq2", name="sq2")
            rx = wt("rx"); t1 = wt("t1"); t2 = wt("t2")
            ph1 = wt("ph1"); ph2 = wt("ph2")
            phi = wt("phi"); corr = wt("corr"); iy = wt("iy")
            cs = wt("cs"); sn = wt("sn"); xc = wt("xc"); ys = wt("ys")
            rr = wt("rr")
            wyab = pool.tile([128, 6, FREE], f32, tag="wyab", name="wyab")
            wya = pool.tile([128, 8, FREE], f16, tag="wya", name="wya")
            wxa = pool.tile([128, 4, FREE], f16, tag="wxa", name="wxa")
            a_t = pool.tile([128, NCELL + 1, NCH, M_LOC], f16, tag="a_t",
                            name="a_t")
            att = a_t[:, NCELL].rearrange("p c m -> p (c m)")  # fp16 psi col

            relx = rel[:, 0].rearrange("p c m -> p (c m)")
            rely = rel[:, 1].rearrange("p c m -> p (c m)")

            V.tensor_tensor(rel[:, 0], fx_b, cb_x, Alu.subtract)
            V.tensor_tensor(rel[:, 1], fy_b, cb_y, Alu.subtract)
            S.activation(sq2[:].rearrange("p a f -> p (a f)"),
                         rel[:].rearrange("p a c m -> p (a c m)"), Act.Square)

            # att = relu(1 - rho)^3 straight into the psi column (fp16)
            CD(ops["EQ_ATT_CUBE2"], out=att, in0=sq2[:, 0], in1=sq2[:, 1])

            # atan2 chain: dual arctan + single merge op
            V.reciprocal_approx_fast(rx[:], relx)
            V.tensor_tensor(t1[:], rely, rx[:], Alu.mult)
            V.reciprocal_approx_fast(t2[:], t1[:])
            S.activation(ph1[:], t1[:], Act.Arctan)
            S.activation(ph2[:], t2[:], Act.Arctan)
            CD(ops["EQ_PHI_MERGE"], out=phi[:], in0=ph1[:], in1=ph2[:],
               s0=math.pi / 2, s1=(math.pi / 4) ** 2)
            CD(ops["EQ_CORR"], out=corr[:], in0=rely, in1=relx)
            CD(ops["EQ_IY_AFF"], out=iy[:], in0=phi[:], in1=corr[:],
               s0=4.0 / math.pi, s1=4.5, imm2=4.0)

            # PE warm-up: tiny throwaway matmuls chained to the elementwise
            # chain keep the tensor engine out of its low p-state; o2t is
            # reset by the first real mm2 accumulation (start=True).
            o2t = psum.tile([16, M_LOC], f32, tag="o2t", name="o2t")
            for wi, dep in enumerate([sq2[:, 0], t1[:], ph1[:], ph2[:],
                                      phi[:], iy[:]][:cfg["pe_warm"]]):
                nc.tensor.matmul(o2t[:, 0:8], inp_s[:, 0:16], dep[:, 0:8],
                                 start=True, stop=True)

            # r = |x cos(phi) + y sin(phi)|
            S.activation(cs[:], phi[:], Act.Sin, bias=math.pi / 2)
            S.activation(sn[:], phi[:], Act.Sin)
            V.tensor_tensor(xc[:], relx, cs[:], Alu.mult)
            V.tensor_tensor(ys[:], rely, sn[:], Alu.mult)
            CD(ops["EQ_RR_ABS"], out=rr[:], in0=xc[:], in1=ys[:])

            # Wx hats with att folded in (fp16): custom 1-op each
            for j in range(4):
                CD(ops["EQ_WX_HAT_A"], out=wxa[:, j], in0=rr[:], in1=att,
                   s0=4.0, s1=-(0.5 + j))

            # Wy bins (pure hats): custom wrap-hat or 2 ACT ops
            ai = 0
            for b8 in range(8):
                cb = float(b8 + 1)
                if b8 in cfg["wy_custom"]:
                    CD(ops["EQ_WRAP_HAT_P"], out=wya[:, b8], in0=iy[:],
                       s0=-cb, s1=8.0)
                else:
                    S.activation(wyab[:, ai], iy[:], Act.Abs, bias=-cb)
                    S.activation(wya[:, b8], wyab[:, ai], Act.Relu,
                                 bias=1.0, scale=-1.0)
                    ai += 1

            # A-cell products (fp16 stock TT); Pool takes the j==3 column
            # of each bin so its slow ops interleave instead of tailing.
            for cell in range(NCELL):
                b8, j = divmod(cell, 4)
                eng = G if j in cfg["pool_j"] else V
                eng.tensor_tensor(a_t[:, cell].rearrange("p c m -> p (c m)"),
                                  wxa[:, j], wya[:, b8], Alu.mult)

            # ---------- matmul 1: G = feat^T @ A ----------
            # group 0 first = cells 30,31 + att column (psi) so psi is ready
            # early; groups of 5 cells fill one PSUM bank each.
            groups = [(30, 33), (0, 5), (5, 10), (10, 15), (15, 20),
                      (20, 25), (25, 30)]
            g_ps = []
            for gi, (c0, c1) in enumerate(groups):
                g_ps.append(psum.tile([48, (c1 - c0) * M_LOC], f32,
                                      tag=f"g{gi}", name=f"g{gi}"))
            for gi, (c0, c1) in enumerate(groups):
                for u in range(NCH):
                    nc.tensor.matmul(g_ps[gi][:],
                                     feat_s[:, u * 48:(u + 1) * 48],
                                     a_t[:, c0:c1, u, :],
                                     start=(u == 0), stop=(u == NCH - 1))

            # ---------- psi -> 1/psi on oy partitions (rows 16..31 of the
            # att column are sum_n mask*att thanks to the ones-pad feat cols)
            psir = pool.tile([16, M_LOC], f32, tag="psir", name="psir")
            V.tensor_scalar(psir[:], g_ps[0][32:48, 2 * M_LOC:3 * M_LOC],
                            1e-30, None, Alu.max)
            V.reciprocal_approx_fast(psir[:], psir[:])

            # ---------- G PSUM -> SBUF (fp16) ----------
            gs = pool.tile([16, NCELL * M_LOC], f16, tag="gs", name="gs")
            # GPSIMD cannot read PSUM: copies go to DVE/ACT only
            copy_engs = [{"S": S, "V": V}[c] for c in cfg["gs_copy_engs"]]
            for gi, (c0, c1) in enumerate(groups):
                w = (min(c1, NCELL) - c0) * M_LOC
                dst = gs[:, c0 * M_LOC:c0 * M_LOC + w]
                e = copy_engs[gi]
                if e is S:
                    S.activation(dst, g_ps[gi][0:16, 0:w], Act.Copy)
                else:
                    e.tensor_copy(dst, g_ps[gi][0:16, 0:w])

            # ---------- matmul 2: out2[oy, m] = sum_cell k2_cell^T @ G_cell
            cell_order = [c for (c0, c1) in groups for c in range(c0, min(c1, NCELL))]
            for i, c in enumerate(cell_order):
                nc.tensor.matmul(o2t[:],
                                 k2_s[:, c * 16:(c + 1) * 16],
                                 gs[:, c * M_LOC:(c + 1) * M_LOC],
                                 start=(i == 0), stop=(i == NCELL - 1))

            # ---------- scale by 1/psi, store [16, 96] ----------
            out_s = pool.tile([16, M_LOC], f32, tag="outs", name="out_s")
            V.tensor_tensor(out_s[:], o2t[:], psir[:], Alu.mult)
            nc.sync.dma_start(outd[:], out_s[:])

    nc.compile()
    return nc


def get_module(cfg=None):
    cfg = dict(CFG, **(cfg or {}))
    key = tuple(sorted((k, str(v)) for k, v in cfg.items()))
    if key not in _module_cache:
        _module_cache[key] = _build_module(cfg)
    return _module_cache[key]


def make_in_maps(field, center, field_feat, field_mask, kernel, cfg=None):
    """Host-side shard + layout prep. Returns list of 8 in_maps."""
    field = np.asarray(field, np.float32)
    center = np.asarray(center, np.float32)
    feat = np.asarray(field_feat, np.float32)
    mask = np.asarray(field_mask, np.float32)
    ker = np.asarray(kernel, np.float32)

    # K2: [16 rows = (ci,x), 512 cols = cell*16 + (co,y)], cell = (theta, r)
    kk = ker.transpose(3, 2, 1, 5, 0, 4).reshape(NCELL, 16, 16)
    k2b = np.ascontiguousarray(
        kk.transpose(1, 0, 2).reshape(16, NCELL * 16)).astype(np.float16)

    in_maps = []
    for c in range(N_CORES):
        b, blk = divmod(c, 4)
        m0 = blk * M_LOC
        cx = center[b, m0:m0 + M_LOC, 0] / RADIUS   # [96]
        cy = center[b, m0:m0 + M_LOC, 1] / RADIUS
        fx = (field[b, :, 0] / RADIUS).reshape(NCH, 128).T  # [128, 3]
        fy = (field[b, :, 1] / RADIUS).reshape(NCH, 128).T

        cst_row = np.array([-l for l in range(10)]
                           + [-(0.5 + j) for j in range(4)]
                           + [math.pi / 2, 1.0], np.float32)
        inp = np.concatenate([np.broadcast_to(cx, (128, M_LOC)),
                              np.broadcast_to(cy, (128, M_LOC)),
                              fx, fy,
                              np.broadcast_to(cst_row, (128, 16))], axis=1)

        fm = feat[b].reshape(N, 16) * mask[b]           # mask folded
        # [N, 48]: 16 feat, 16 zero, 16 mask (psi lands on PSUM rows 32..47,
        # which the scalar engine can address: 32-aligned partition window)
        fcols = np.concatenate(
            [fm, np.zeros((N, 16), np.float32),
             np.broadcast_to(mask[b], (N, 16))], axis=1)
        featx = fcols.reshape(NCH, 128, 48).transpose(1, 0, 2).reshape(128, 144)

        in_maps.append({
            "inp": np.ascontiguousarray(inp, np.float32),
            "featx": np.ascontiguousarray(featx).astype(np.float16),
            "k2b": k2b,
        })
    return in_maps


def unshard(results):
    out = np.zeros((B, M, CO, 2), np.float32)
    for c in range(N_CORES):
        b, blk = divmod(c, 4)
        m0 = blk * M_LOC
        # core out is [16 = (co, y), 96 m]
        o = results[c]["out"].reshape(CO, 2, M_LOC).transpose(2, 0, 1)
        out[b, m0:m0 + M_LOC] = o
    return out


def kernel(field, center, field_feat, field_mask, kernel):
    from concourse.bass_utils import run_bass_kernel_spmd
    nc = get_module()
    in_maps = make_in_maps(field, center, field_feat, field_mask, kernel)
    res = run_bass_kernel_spmd(nc, in_maps, core_ids=list(range(N_CORES)))
    return unshard(res.results)
